# revision 11
# baseline (speedup 1.0000x reference)
"""CoAttention ImageDNS kernel for Trainium2 (8 NeuronCores, Bass/Tile).

Math: the reference computes two additive-attention blocks. In both, the
softmax'd score is  score[b, q, k] = f(q-side)[b, q] + g(k-side)[b, k] + c,
and softmax over k is invariant to the q-dependent (and constant) terms, so
the attention weights are independent of the query index:

  visual_att[b, s, :]  = softmax_r( wB . tanh(W_i1 @ img[b, r]) )
  textual_att[b, i, :] = softmax_j( wD . tanh(W_d2 @ dns[b, j]) )

Hence both outputs are per-batch rank-1 broadcasts:

  att_img_features[b, s, :] = visual_att[b]  @ img[b]   (same for all s)
  att_dns_features[b, i, :] = textual_att[b] @ dns[b]   (same for all i)

W_d1/b_d1/w_att1[:H]/b_att1/W_i2/b_i2/w_att2[:H]/b_att2 cancel entirely.

Sharding: pure data-parallel over batch, 4 batches per core, no collectives.

Perf notes vs the bf16 baseline (140.2us):
- Projection h-blocks 0..3 run as fp8(e4m3) DoubleRow matmuls with a REAL
  256-deep contraction per matmul (2 h-blocks per pair-column), which the PE
  streams at the same column rate as a 128-deep bf16 matmul -> 2x throughput
  on that half. Blocks 4..7 stay bf16. Net projection cost 0.75x, end-to-end
  rel err ~1.77e-2 vs the 2e-2 gate (fp8 on ALL blocks would be 2.6e-2; W is
  pre-scaled by 64 so its entries clear e4m3's subnormal floor). The
  DoubleRow pair-dim AP step must be 16B-aligned, hence img rows pad to 208.
- The tiny score-sum and stage-2 matmuls for item k are emitted AFTER all of
  item k+1's projection matmuls, so the PE stream never waits on the
  scalar/vector tanh/score chain (PE idle gaps re-throttle the HAM clock
  gate from 2.4GHz to 1.2GHz, which is what capped the baseline).
- All DRAM operands are partition-major so every tile is one DMA with
  2-8KB per-partition lines (small lines gate early DMA throughput), spread
  over three queues: weights on Sync's, batch-0 x tiles on Activation's,
  the rest on GpSimd's/Activation's, ordered by first use.
- Chunk pairs emit their fp8 groups back-to-back to halve PE mode switches.
- Only one [1, H] output row per (batch, side) leaves the device; the
  broadcast over S is done on host (kills 16MB/core of output DMA).
- Stage 2 (attention-weighted sum of rows) stays bf16: e4m3 there would put
  ~3.6% error directly on the output.
"""

import sys
import numpy as np
import ml_dtypes

_BF16 = ml_dtypes.bfloat16
_E4M3 = ml_dtypes.float8_e4m3

for _p in ("/opt/trn_rl_repo", "/root/.axon_site/_ro/trn_rl_repo"):
    if _p not in sys.path:
        sys.path.append(_p)

B, S, R, H = 32, 512, 196, 1024
NCORES = 8
BLOC = B // NCORES          # batches per core
OC = 512                    # output-chunk (one fp32 PSUM bank)
NB8 = 4                     # h-blocks 0..3 in e4m3 (2 DoubleRow matmuls)
NBB = 4                     # h-blocks 4..7 in bf16
WSCALE = 64.0               # W pre-scale so e4m3 entries are normal numbers
R8 = 208                    # img rows padded for 16B-aligned DoubleRow steps

_CACHE = {}


def _row_chunks(n):
    out, o = [], 0
    while o < n:
        out.append((o, min(128, n - o)))
        o += 128
    return out


def build_nc():
    from concourse import bacc, mybir
    from concourse import tile

    f32, f16, f8 = mybir.dt.float32, mybir.dt.bfloat16, mybir.dt.float8e4
    Act = mybir.ActivationFunctionType
    Alu = mybir.AluOpType
    DR = mybir.MatmulPerfMode.DoubleRow

    nc = bacc.Bacc("TRN2", target_bir_lowering=False, debug=False)

    x8_dns = nc.dram_tensor("x8_dns", [BLOC, 128, NB8 * S], f8, kind="ExternalInput")
    x8_img = nc.dram_tensor("x8_img", [BLOC, 128, NB8 * R8], f8, kind="ExternalInput")
    xb_dns = nc.dram_tensor("xb_dns", [BLOC, 128, NBB * S], f16, kind="ExternalInput")
    xb_img = nc.dram_tensor("xb_img", [BLOC, 128, NBB * R], f16, kind="ExternalInput")
    xn_dns = nc.dram_tensor("xn_dns", [BLOC, 128, 4 * H], f16, kind="ExternalInput")
    xn_img = nc.dram_tensor("xn_img", [BLOC, 128, 2 * H], f16, kind="ExternalInput")
    w8_i1 = nc.dram_tensor("w8_i1", [128, NB8 * H], f8, kind="ExternalInput")
    wb_i1 = nc.dram_tensor("wb_i1", [128, NBB * H], f16, kind="ExternalInput")
    w8_d2 = nc.dram_tensor("w8_d2", [128, NB8 * H], f8, kind="ExternalInput")
    wb_d2 = nc.dram_tensor("wb_d2", [128, NBB * H], f16, kind="ExternalInput")
    wrow_b = nc.dram_tensor("wrow_b", [128, H], f32, kind="ExternalInput")
    wrow_d = nc.dram_tensor("wrow_d", [128, H], f32, kind="ExternalInput")
    out_rows = nc.dram_tensor("out_rows", [BLOC, 2, H], f32, kind="ExternalOutput")

    with tile.TileContext(nc) as tc:
        with (
            tc.tile_pool(name="const", bufs=1) as cpool,
            tc.tile_pool(name="xts", bufs=2) as xtpool,
            tc.tile_pool(name="xns", bufs=2) as xnpool,
            tc.tile_pool(name="work", bufs=3) as wpool,
            tc.tile_pool(name="small", bufs=2) as spool,
            tc.tile_pool(name="outs", bufs=2) as opool,
            tc.tile_pool(name="pp", bufs=2, space="PSUM") as ppool,
            tc.tile_pool(name="pa", bufs=2, space="PSUM") as papool,
            tc.tile_pool(name="ps", bufs=2, space="PSUM") as pstat,
        ):
            wt_sb, wrow_sb = {}, {}

            def get_wrow(nm):
                if nm not in wrow_sb:
                    dram = {"b": wrow_b, "d": wrow_d}[nm]
                    w = cpool.tile([128, H], f32, name=f"wrow_{nm}_sb")
                    nc.gpsimd.dma_start(out=w[:, :], in_=dram[:, :])
                    wrow_sb[nm] = w
                return wrow_sb[nm]

            ones_col = cpool.tile([128, 1], f16, name="ones_col")
            nc.vector.memset(ones_col[:, :], 1.0)

            def emit_proj(b, side):
                n_rows = R if side == "img" else S
                n8 = R8 if side == "img" else S
                x8_d = x8_img if side == "img" else x8_dns
                xb_d = xb_img if side == "img" else xb_dns
                xn_d = xn_img if side == "img" else xn_dns
                wt_name = "i1" if side == "img" else "d2"
                # weights on the Sync queue (first to start); batch-0 x tiles
                # on Activation's queue so they flow in parallel with weights;
                # later batches prefetch on GpSimd's queue
                xq = nc.scalar if b == 0 else nc.gpsimd
                load_wt = wt_name not in wt_sb
                if load_wt:
                    w8_d, wb_d = (w8_i1, wb_i1) if side == "img" else (w8_d2, wb_d2)
                    w8 = cpool.tile([128, NB8 * H], f8, name=f"w8_{wt_name}_sb")
                    wb = cpool.tile([128, NBB * H], f16, name=f"wb_{wt_name}_sb")
                    nc.sync.dma_start(out=w8[:, :], in_=w8_d[:, :])
                    nc.sync.dma_start(out=wb[:, :], in_=wb_d[:, :])
                    wt_sb[wt_name] = (w8, wb)
                w8, wb = wt_sb[wt_name]
                w8v = w8.rearrange("p (j o) -> p j o", j=NB8)
                rcs = _row_chunks(n_rows)

                x8_t = xtpool.tile([128, NB8 * n8], f8,
                                   name=f"x8_{side}_{b}", tag=f"x8_{side}")
                xq.dma_start(out=x8_t[:, :], in_=x8_d[b])
                xb_t = xtpool.tile([128, NBB * n_rows], f16,
                                   name=f"xb_{side}_{b}", tag=f"xb_{side}")
                xq.dma_start(out=xb_t[:, :], in_=xb_d[b])
                x8v = x8_t.rearrange("p (j m) -> p j m", j=NB8)

                acols = []
                xn_ts = []
                wr = None

                def emit_dr(ci, r0, rk):
                    ps = ppool.tile([128, H], f32, name=f"proj_{side}_{ci}_{b}",
                                    tag="pp")
                    for u in range(NB8 // 2):
                        lhs = x8v[:, 2 * u:2 * u + 2, r0:r0 + rk]
                        for oc in range(2):
                            nc.tensor.matmul(
                                ps[0:rk, oc * OC:(oc + 1) * OC],
                                lhsT=lhs,
                                rhs=w8v[:, 2 * u:2 * u + 2, oc * OC:(oc + 1) * OC],
                                start=(u == 0), stop=False,
                                perf_mode=DR)
                    return ps

                def emit_bf(ci, r0, rk, ps):
                    for j in range(NBB):
                        lhs = xb_t[:, j * n_rows + r0: j * n_rows + r0 + rk]
                        for oc in range(2):
                            nc.tensor.matmul(
                                ps[0:rk, oc * OC:(oc + 1) * OC],
                                lhsT=lhs,
                                rhs=wb[:, j * H + oc * OC: j * H + (oc + 1) * OC],
                                start=False, stop=(j == NBB - 1))

                def emit_act(ci, r0, rk, ps):
                    th = wpool.tile([128, H], f32, name=f"th_{side}_{ci}_{b}",
                                    tag="th")
                    nc.scalar.activation(th[0:rk, :], ps[0:rk, :], Act.Tanh,
                                         scale=1.0 / WSCALE)
                    scr = wpool.tile([128, H], f32, name=f"scr_{side}_{ci}_{b}",
                                     tag="scr", bufs=2)
                    tcol = spool.tile([128, 1], f32, name=f"tc_{side}_{ci}_{b}",
                                      tag="tcol", bufs=3)
                    nc.vector.scalar_tensor_tensor(
                        out=scr[0:rk, :], in0=th[0:rk, :], scalar=1.0,
                        in1=wr[0:rk, :], op0=Alu.mult, op1=Alu.mult,
                        accum_out=tcol[0:rk, :])
                    acol = spool.tile([128, 1], f16, name=f"a_{side}_{ci}_{b}",
                                      tag=f"acol_{side}_{ci}", bufs=2)
                    nc.scalar.activation(acol[0:rk, :], tcol[0:rk, :], Act.Exp)
                    acols.append((acol, rk))

                # chunk pairs: fp8 groups of both chunks back-to-back, halving
                # PE fp8<->bf16 mode switches
                for c0 in range(0, len(rcs), 2):
                    pair = [(ci, rcs[ci]) for ci in range(c0, min(c0 + 2, len(rcs)))]
                    pss = [emit_dr(ci, r0, rk) for ci, (r0, rk) in pair]
                    if c0 == 0:
                        nrc = len(rcs)
                        xn_t = xnpool.tile([128, nrc * H], f16,
                                           name=f"xn_{side}_{b}", tag=f"xn_{side}")
                        # stage-2 activations are consumed one pipeline item
                        # later; batch 0's ride GpSimd's queue, the rest
                        # Activation's
                        xnq = nc.gpsimd if b == 0 else nc.scalar
                        xnq.dma_start(out=xn_t[:, :], in_=xn_d[b])
                        xn_ts = [xn_t[:, cj * H:(cj + 1) * H] for cj in range(nrc)]
                        wr = get_wrow("b" if side == "img" else "d")
                    for (ci, (r0, rk)), ps in zip(pair, pss):
                        emit_bf(ci, r0, rk, ps)
                    for (ci, (r0, rk)), ps in zip(pair, pss):
                        emit_act(ci, r0, rk, ps)
                return (b, side, acols, xn_ts)

            def emit_reduce(state):
                b, side, acols, xn_ts = state
                sd = 0 if side == "img" else 1
                s_ps = pstat.tile([1, 1], f32, name=f"s_{side}_{b}", tag="stat")
                for ci, (acol, rk) in enumerate(acols):
                    nc.tensor.matmul(
                        s_ps[0:1, 0:1], lhsT=acol[0:rk, 0:1],
                        rhs=ones_col[0:rk, 0:1],
                        start=(ci == 0), stop=(ci == len(acols) - 1))
                r_sb = spool.tile([1, 1], f32, name=f"r_{side}_{b}", tag="r", bufs=2)
                nc.vector.reciprocal(r_sb[0:1, 0:1], s_ps[0:1, 0:1])
                att_sb = opool.tile([1, H], f32, name=f"attsb_{side}_{b}",
                                    tag="att")
                for oc in range(2):
                    att_ps = papool.tile([1, OC], f32,
                                         name=f"att_{side}_{b}_{oc}", tag="attps")
                    for ci, (acol, rk) in enumerate(acols):
                        nc.tensor.matmul(
                            att_ps[0:1, :],
                            lhsT=acol[0:rk, 0:1],
                            rhs=xn_ts[ci][0:rk, oc * OC:(oc + 1) * OC],
                            start=(ci == 0), stop=(ci == len(acols) - 1))
                    nc.scalar.activation(att_sb[0:1, oc * OC:(oc + 1) * OC],
                                         att_ps[0:1, :],
                                         Act.Copy, scale=r_sb[0:1, 0:1])
                nc.sync.dma_start(out=out_rows[b, sd:sd + 1, :],
                                  in_=att_sb[0:1, :])

            pending = None
            for b in range(BLOC):
                for side in ("img", "dns"):
                    state = emit_proj(b, side)
                    if pending is not None:
                        emit_reduce(pending)
                    pending = state
            emit_reduce(pending)
    nc.compile()
    return nc


def _get_nc():
    if "nc" not in _CACHE:
        _CACHE["nc"] = build_nc()
    return _CACHE["nc"]


def make_in_maps(inputs):
    dns = np.ascontiguousarray(np.asarray(inputs["dns_feature"], dtype=np.float32))
    img = np.ascontiguousarray(np.asarray(inputs["img_features"], dtype=np.float32))
    W_i1 = np.asarray(inputs["W_i1"], dtype=np.float32)
    W_d2 = np.asarray(inputs["W_d2"], dtype=np.float32)
    wB = np.asarray(inputs["w_att1"], dtype=np.float32)[H:]
    wD = np.asarray(inputs["w_att2"], dtype=np.float32)[H:]

    def pack_w(W):
        wt = np.ascontiguousarray(W.T) * WSCALE         # [h_in, o]
        w8 = np.ascontiguousarray(
            wt[:NB8 * 128].reshape(NB8, 128, H).transpose(1, 0, 2)
            .reshape(128, NB8 * H)).astype(_E4M3)
        wb = np.ascontiguousarray(
            wt[NB8 * 128:].reshape(NBB, 128, H).transpose(1, 0, 2)
            .reshape(128, NBB * H)).astype(_BF16)
        return w8, wb
    w8_i1, wb_i1 = pack_w(W_i1)
    w8_d2, wb_d2 = pack_w(W_d2)
    wrow_b = np.ascontiguousarray(np.broadcast_to(wB, (128, H)))
    wrow_d = np.ascontiguousarray(np.broadcast_to(wD, (128, H)))

    def pack_x(x, n, n8):
        xt = x.transpose(0, 2, 1).reshape(B, 8, 128, n)
        x8 = np.zeros((B, NB8, 128, n8), dtype=_E4M3)
        x8[:, :, :, :n] = xt[:, :NB8].astype(_E4M3)
        x8 = np.ascontiguousarray(x8.transpose(0, 2, 1, 3).reshape(B, 128, NB8 * n8))
        xb = xt[:, NB8:].astype(_BF16)
        xb = np.ascontiguousarray(xb.transpose(0, 2, 1, 3).reshape(B, 128, NBB * n))
        return x8, xb
    x8_dns, xb_dns = pack_x(dns, S, S)
    x8_img, xb_img = pack_x(img, R, R8)

    def pack_xn(x, nrc):
        xp = np.zeros((B, nrc * 128, H), dtype=np.float32)
        xp[:, :x.shape[1]] = x
        return np.ascontiguousarray(
            xp.reshape(B, nrc, 128, H).transpose(0, 2, 1, 3)
            .reshape(B, 128, nrc * H)).astype(_BF16)
    xn_dns = pack_xn(dns, 4)
    xn_img = pack_xn(img, 2)

    in_maps = []
    for k in range(NCORES):
        sl = slice(k * BLOC, (k + 1) * BLOC)
        in_maps.append({
            "x8_dns": np.ascontiguousarray(x8_dns[sl]),
            "x8_img": np.ascontiguousarray(x8_img[sl]),
            "xb_dns": np.ascontiguousarray(xb_dns[sl]),
            "xb_img": np.ascontiguousarray(xb_img[sl]),
            "xn_dns": np.ascontiguousarray(xn_dns[sl]),
            "xn_img": np.ascontiguousarray(xn_img[sl]),
            "w8_i1": w8_i1, "wb_i1": wb_i1,
            "w8_d2": w8_d2, "wb_d2": wb_d2,
            "wrow_b": wrow_b, "wrow_d": wrow_d,
        })
    return in_maps


def kernel(**inputs):
    from concourse.bass_utils import run_bass_kernel_spmd

    nc = _get_nc()
    in_maps = make_in_maps(inputs)
    res = run_bass_kernel_spmd(nc, in_maps, list(range(NCORES))).results
    rows = np.concatenate([res[k]["out_rows"] for k in range(NCORES)], axis=0)
    att_img = np.ascontiguousarray(
        np.broadcast_to(rows[:, 0][:, None, :], (B, S, H)))
    att_dns = np.ascontiguousarray(
        np.broadcast_to(rows[:, 1][:, None, :], (B, S, H)))
    return att_dns, att_img


# revision 12
# speedup vs baseline: 1.1101x; 1.1101x over previous
"""CoAttention ImageDNS kernel for Trainium2 (8 NeuronCores, Bass/Tile).

Math: the reference computes two additive-attention blocks. In both, the
softmax'd score is  score[b, q, k] = f(q-side)[b, q] + g(k-side)[b, k] + c,
and softmax over k is invariant to the q-dependent (and constant) terms, so
the attention weights are independent of the query index:

  visual_att[b, s, :]  = softmax_r( wB . tanh(W_i1 @ img[b, r]) )
  textual_att[b, i, :] = softmax_j( wD . tanh(W_d2 @ dns[b, j]) )

Hence both outputs are per-batch rank-1 broadcasts:

  att_img_features[b, s, :] = visual_att[b]  @ img[b]   (same for all s)
  att_dns_features[b, i, :] = textual_att[b] @ dns[b]   (same for all i)

W_d1/b_d1/w_att1[:H]/b_att1/W_i2/b_i2/w_att2[:H]/b_att2 cancel entirely.

Sharding: pure data-parallel over batch, 4 batches per core, no collectives.

Perf notes vs the bf16 baseline (140.2us):
- Projection h-blocks 0..3 run as fp8(e4m3) DoubleRow matmuls with a REAL
  256-deep contraction per matmul (2 h-blocks per pair-column), which the PE
  streams at the same column rate as a 128-deep bf16 matmul -> 2x throughput
  on that half. Blocks 4..7 stay bf16. Net projection cost 0.75x, end-to-end
  rel err ~1.77e-2 vs the 2e-2 gate (fp8 on ALL blocks would be 2.6e-2; W is
  pre-scaled by 64 so its entries clear e4m3's subnormal floor). The
  DoubleRow pair-dim AP step must be 16B-aligned, hence img rows pad to 208.
- The tiny score-sum and stage-2 matmuls for item k are emitted AFTER all of
  item k+1's projection matmuls, so the PE stream never waits on the
  scalar/vector tanh/score chain (PE idle gaps re-throttle the HAM clock
  gate from 2.4GHz to 1.2GHz, which is what capped the baseline).
- All DRAM operands are partition-major so every tile is one DMA with
  2-8KB per-partition lines (small lines gate early DMA throughput), spread
  over three queues: weights on Sync's, batch-0 x tiles on Activation's,
  the rest on GpSimd's/Activation's, ordered by first use.
- Chunk pairs emit their fp8 groups back-to-back to halve PE mode switches.
- Only one [1, H] output row per (batch, side) leaves the device; the
  broadcast over S is done on host (kills 16MB/core of output DMA).
- Stage 2 (attention-weighted sum of rows) stays bf16: e4m3 there would put
  ~3.6% error directly on the output.
"""

import sys
import numpy as np
import ml_dtypes

_BF16 = ml_dtypes.bfloat16
_E4M3 = ml_dtypes.float8_e4m3

for _p in ("/opt/trn_rl_repo", "/root/.axon_site/_ro/trn_rl_repo"):
    if _p not in sys.path:
        sys.path.append(_p)

B, S, R, H = 32, 512, 196, 1024
NCORES = 8
BLOC = B // NCORES          # batches per core
OC = 512                    # output-chunk (one fp32 PSUM bank)
NB8 = 4                     # h-blocks 0..3 in e4m3 (2 DoubleRow matmuls)
NBB = 4                     # h-blocks 4..7 in bf16
WSCALE = 64.0               # W pre-scale so e4m3 entries are normal numbers
R8 = 208                    # img rows padded for 16B-aligned DoubleRow steps

_CACHE = {}


def _row_chunks(n):
    out, o = [], 0
    while o < n:
        out.append((o, min(128, n - o)))
        o += 128
    return out


def build_nc():
    from concourse import bacc, mybir
    from concourse import tile

    f32, f16, f8 = mybir.dt.float32, mybir.dt.bfloat16, mybir.dt.float8e4
    Act = mybir.ActivationFunctionType
    Alu = mybir.AluOpType
    DR = mybir.MatmulPerfMode.DoubleRow

    nc = bacc.Bacc("TRN2", target_bir_lowering=False, debug=False)

    x8_dns = nc.dram_tensor("x8_dns", [BLOC, 128, NB8 * S], f8, kind="ExternalInput")
    x8_img = nc.dram_tensor("x8_img", [BLOC, 128, NB8 * R8], f8, kind="ExternalInput")
    xb_dns = nc.dram_tensor("xb_dns", [BLOC, 128, NBB * S], f16, kind="ExternalInput")
    xb_img = nc.dram_tensor("xb_img", [BLOC, 128, NBB * R], f16, kind="ExternalInput")
    xn_dns = nc.dram_tensor("xn_dns", [BLOC, 128, 4 * H], f16, kind="ExternalInput")
    xn_img = nc.dram_tensor("xn_img", [BLOC, 128, 2 * H], f16, kind="ExternalInput")
    w8_i1 = nc.dram_tensor("w8_i1", [128, NB8 * H], f8, kind="ExternalInput")
    wb_i1 = nc.dram_tensor("wb_i1", [128, NBB * H], f16, kind="ExternalInput")
    w8_d2 = nc.dram_tensor("w8_d2", [128, NB8 * H], f8, kind="ExternalInput")
    wb_d2 = nc.dram_tensor("wb_d2", [128, NBB * H], f16, kind="ExternalInput")
    wrow_b = nc.dram_tensor("wrow_b", [128, H], f32, kind="ExternalInput")
    wrow_d = nc.dram_tensor("wrow_d", [128, H], f32, kind="ExternalInput")
    out_rows = nc.dram_tensor("out_rows", [BLOC, 2, H], f32, kind="ExternalOutput")

    with tile.TileContext(nc) as tc:
        with (
            tc.tile_pool(name="const", bufs=1) as cpool,
            tc.tile_pool(name="xts", bufs=2) as xtpool,
            tc.tile_pool(name="xns", bufs=2) as xnpool,
            tc.tile_pool(name="work", bufs=3) as wpool,
            tc.tile_pool(name="small", bufs=2) as spool,
            tc.tile_pool(name="outs", bufs=2) as opool,
            tc.tile_pool(name="pp", bufs=2, space="PSUM") as ppool,
            tc.tile_pool(name="pa", bufs=2, space="PSUM") as papool,
            tc.tile_pool(name="ps", bufs=2, space="PSUM") as pstat,
        ):
            wt_sb, wrow_sb = {}, {}

            def get_wrow(nm):
                if nm not in wrow_sb:
                    dram = {"b": wrow_b, "d": wrow_d}[nm]
                    w = cpool.tile([128, H], f32, name=f"wrow_{nm}_sb")
                    nc.scalar.dma_start(out=w[:, :], in_=dram[:, :])
                    wrow_sb[nm] = w
                return wrow_sb[nm]

            ones_col = cpool.tile([128, 1], f16, name="ones_col")
            nc.vector.memset(ones_col[:, :], 1.0)

            def emit_proj(b, side):
                n_rows = R if side == "img" else S
                n8 = R8 if side == "img" else S
                x8_d = x8_img if side == "img" else x8_dns
                xb_d = xb_img if side == "img" else xb_dns
                xn_d = xn_img if side == "img" else xn_dns
                wt_name = "i1" if side == "img" else "d2"
                # batch-0 tiles + first-use weights ride the Sync queue (the
                # first to start) in consumption order; later batches'
                # x tiles prefetch on GpSimd's queue
                xq = nc.sync if b == 0 else nc.gpsimd
                load_wt = wt_name not in wt_sb
                if load_wt:
                    w8_d, wb_d = (w8_i1, wb_i1) if side == "img" else (w8_d2, wb_d2)
                    w8 = cpool.tile([128, NB8 * H], f8, name=f"w8_{wt_name}_sb")
                    wb = cpool.tile([128, NBB * H], f16, name=f"wb_{wt_name}_sb")
                    nc.sync.dma_start(out=w8[:, :], in_=w8_d[:, :])
                    nc.sync.dma_start(out=wb[:, :], in_=wb_d[:, :])
                    wt_sb[wt_name] = (w8, wb)
                w8, wb = wt_sb[wt_name]
                w8v = w8.rearrange("p (j o) -> p j o", j=NB8)
                rcs = _row_chunks(n_rows)

                x8_t = xtpool.tile([128, NB8 * n8], f8,
                                   name=f"x8_{side}_{b}", tag=f"x8_{side}")
                xq.dma_start(out=x8_t[:, :], in_=x8_d[b])
                xb_t = xtpool.tile([128, NBB * n_rows], f16,
                                   name=f"xb_{side}_{b}", tag=f"xb_{side}")
                xq.dma_start(out=xb_t[:, :], in_=xb_d[b])
                x8v = x8_t.rearrange("p (j m) -> p j m", j=NB8)

                acols = []
                xn_ts = []
                wr = None

                def emit_dr(ci, r0, rk):
                    ps = ppool.tile([128, H], f32, name=f"proj_{side}_{ci}_{b}",
                                    tag="pp")
                    for u in range(NB8 // 2):
                        lhs = x8v[:, 2 * u:2 * u + 2, r0:r0 + rk]
                        for oc in range(2):
                            nc.tensor.matmul(
                                ps[0:rk, oc * OC:(oc + 1) * OC],
                                lhsT=lhs,
                                rhs=w8v[:, 2 * u:2 * u + 2, oc * OC:(oc + 1) * OC],
                                start=(u == 0), stop=False,
                                perf_mode=DR)
                    return ps

                def emit_bf(ci, r0, rk, ps):
                    for j in range(NBB):
                        lhs = xb_t[:, j * n_rows + r0: j * n_rows + r0 + rk]
                        for oc in range(2):
                            nc.tensor.matmul(
                                ps[0:rk, oc * OC:(oc + 1) * OC],
                                lhsT=lhs,
                                rhs=wb[:, j * H + oc * OC: j * H + (oc + 1) * OC],
                                start=False, stop=(j == NBB - 1))

                def emit_act(ci, r0, rk, ps):
                    th = wpool.tile([128, H], f32, name=f"th_{side}_{ci}_{b}",
                                    tag="th")
                    nc.scalar.activation(th[0:rk, :], ps[0:rk, :], Act.Tanh,
                                         scale=1.0 / WSCALE)
                    scr = wpool.tile([128, H], f32, name=f"scr_{side}_{ci}_{b}",
                                     tag="scr", bufs=2)
                    tcol = spool.tile([128, 1], f32, name=f"tc_{side}_{ci}_{b}",
                                      tag="tcol", bufs=3)
                    nc.vector.scalar_tensor_tensor(
                        out=scr[0:rk, :], in0=th[0:rk, :], scalar=1.0,
                        in1=wr[0:rk, :], op0=Alu.mult, op1=Alu.mult,
                        accum_out=tcol[0:rk, :])
                    acol = spool.tile([128, 1], f16, name=f"a_{side}_{ci}_{b}",
                                      tag=f"acol_{side}_{ci}", bufs=2)
                    nc.scalar.activation(acol[0:rk, :], tcol[0:rk, :], Act.Exp)
                    acols.append((acol, rk))

                # chunk pairs: fp8 groups of both chunks back-to-back, halving
                # PE fp8<->bf16 mode switches
                for c0 in range(0, len(rcs), 2):
                    pair = [(ci, rcs[ci]) for ci in range(c0, min(c0 + 2, len(rcs)))]
                    pss = [emit_dr(ci, r0, rk) for ci, (r0, rk) in pair]
                    if c0 == 0:
                        nrc = len(rcs)
                        xn_t = xnpool.tile([128, nrc * H], f16,
                                           name=f"xn_{side}_{b}", tag=f"xn_{side}")
                        # stage-2 activations stream on Activation's queue;
                        # they are consumed one pipeline item later
                        nc.scalar.dma_start(out=xn_t[:, :], in_=xn_d[b])
                        xn_ts = [xn_t[:, cj * H:(cj + 1) * H] for cj in range(nrc)]
                        wr = get_wrow("b" if side == "img" else "d")
                    for (ci, (r0, rk)), ps in zip(pair, pss):
                        emit_bf(ci, r0, rk, ps)
                    for (ci, (r0, rk)), ps in zip(pair, pss):
                        emit_act(ci, r0, rk, ps)
                return (b, side, acols, xn_ts)

            def emit_reduce(state):
                b, side, acols, xn_ts = state
                sd = 0 if side == "img" else 1
                s_ps = pstat.tile([1, 1], f32, name=f"s_{side}_{b}", tag="stat")
                for ci, (acol, rk) in enumerate(acols):
                    nc.tensor.matmul(
                        s_ps[0:1, 0:1], lhsT=acol[0:rk, 0:1],
                        rhs=ones_col[0:rk, 0:1],
                        start=(ci == 0), stop=(ci == len(acols) - 1))
                r_sb = spool.tile([1, 1], f32, name=f"r_{side}_{b}", tag="r", bufs=2)
                nc.vector.reciprocal(r_sb[0:1, 0:1], s_ps[0:1, 0:1])
                att_sb = opool.tile([1, H], f32, name=f"attsb_{side}_{b}",
                                    tag="att")
                for oc in range(2):
                    att_ps = papool.tile([1, OC], f32,
                                         name=f"att_{side}_{b}_{oc}", tag="attps")
                    for ci, (acol, rk) in enumerate(acols):
                        nc.tensor.matmul(
                            att_ps[0:1, :],
                            lhsT=acol[0:rk, 0:1],
                            rhs=xn_ts[ci][0:rk, oc * OC:(oc + 1) * OC],
                            start=(ci == 0), stop=(ci == len(acols) - 1))
                    nc.scalar.activation(att_sb[0:1, oc * OC:(oc + 1) * OC],
                                         att_ps[0:1, :],
                                         Act.Copy, scale=r_sb[0:1, 0:1])
                nc.sync.dma_start(out=out_rows[b, sd:sd + 1, :],
                                  in_=att_sb[0:1, :])

            pending = None
            for b in range(BLOC):
                for side in ("img", "dns"):
                    state = emit_proj(b, side)
                    if pending is not None:
                        emit_reduce(pending)
                    pending = state
            emit_reduce(pending)
    nc.compile()
    return nc


def _get_nc():
    if "nc" not in _CACHE:
        _CACHE["nc"] = build_nc()
    return _CACHE["nc"]


def make_in_maps(inputs):
    dns = np.ascontiguousarray(np.asarray(inputs["dns_feature"], dtype=np.float32))
    img = np.ascontiguousarray(np.asarray(inputs["img_features"], dtype=np.float32))
    W_i1 = np.asarray(inputs["W_i1"], dtype=np.float32)
    W_d2 = np.asarray(inputs["W_d2"], dtype=np.float32)
    wB = np.asarray(inputs["w_att1"], dtype=np.float32)[H:]
    wD = np.asarray(inputs["w_att2"], dtype=np.float32)[H:]

    def pack_w(W):
        wt = np.ascontiguousarray(W.T) * WSCALE         # [h_in, o]
        w8 = np.ascontiguousarray(
            wt[:NB8 * 128].reshape(NB8, 128, H).transpose(1, 0, 2)
            .reshape(128, NB8 * H)).astype(_E4M3)
        wb = np.ascontiguousarray(
            wt[NB8 * 128:].reshape(NBB, 128, H).transpose(1, 0, 2)
            .reshape(128, NBB * H)).astype(_BF16)
        return w8, wb
    w8_i1, wb_i1 = pack_w(W_i1)
    w8_d2, wb_d2 = pack_w(W_d2)
    wrow_b = np.ascontiguousarray(np.broadcast_to(wB, (128, H)))
    wrow_d = np.ascontiguousarray(np.broadcast_to(wD, (128, H)))

    def pack_x(x, n, n8):
        xt = x.transpose(0, 2, 1).reshape(B, 8, 128, n)
        x8 = np.zeros((B, NB8, 128, n8), dtype=_E4M3)
        x8[:, :, :, :n] = xt[:, :NB8].astype(_E4M3)
        x8 = np.ascontiguousarray(x8.transpose(0, 2, 1, 3).reshape(B, 128, NB8 * n8))
        xb = xt[:, NB8:].astype(_BF16)
        xb = np.ascontiguousarray(xb.transpose(0, 2, 1, 3).reshape(B, 128, NBB * n))
        return x8, xb
    x8_dns, xb_dns = pack_x(dns, S, S)
    x8_img, xb_img = pack_x(img, R, R8)

    def pack_xn(x, nrc):
        xp = np.zeros((B, nrc * 128, H), dtype=np.float32)
        xp[:, :x.shape[1]] = x
        return np.ascontiguousarray(
            xp.reshape(B, nrc, 128, H).transpose(0, 2, 1, 3)
            .reshape(B, 128, nrc * H)).astype(_BF16)
    xn_dns = pack_xn(dns, 4)
    xn_img = pack_xn(img, 2)

    in_maps = []
    for k in range(NCORES):
        sl = slice(k * BLOC, (k + 1) * BLOC)
        in_maps.append({
            "x8_dns": np.ascontiguousarray(x8_dns[sl]),
            "x8_img": np.ascontiguousarray(x8_img[sl]),
            "xb_dns": np.ascontiguousarray(xb_dns[sl]),
            "xb_img": np.ascontiguousarray(xb_img[sl]),
            "xn_dns": np.ascontiguousarray(xn_dns[sl]),
            "xn_img": np.ascontiguousarray(xn_img[sl]),
            "w8_i1": w8_i1, "wb_i1": wb_i1,
            "w8_d2": w8_d2, "wb_d2": wb_d2,
            "wrow_b": wrow_b, "wrow_d": wrow_d,
        })
    return in_maps


def kernel(**inputs):
    from concourse.bass_utils import run_bass_kernel_spmd

    nc = _get_nc()
    in_maps = make_in_maps(inputs)
    res = run_bass_kernel_spmd(nc, in_maps, list(range(NCORES))).results
    rows = np.concatenate([res[k]["out_rows"] for k in range(NCORES)], axis=0)
    att_img = np.ascontiguousarray(
        np.broadcast_to(rows[:, 0][:, None, :], (B, S, H)))
    att_dns = np.ascontiguousarray(
        np.broadcast_to(rows[:, 1][:, None, :], (B, S, H)))
    return att_dns, att_img
